# revision 37
# baseline (speedup 1.0000x reference)
"""Trainium2 Bass kernel for nn_AttentionAggregator (GNN message passing).

Math (per batch row b, with N=64 neighbors, F=128 in-features, H=8 heads, D=64):
    lin  = x @ W_lin                                      [B, N, 512]
    att  = lin[:,0,:] @ W_att[:512] + lin @ W_att[512:]   [B, N, 8]
    att  = LeakyReLU_0.2(att); masked softmax over N per (b, h)
    out  = relu(lin * aw)                                 [B, N, 512]

Design (v4, tuned against neuron-profile traces):
  * Attention contracts through W_lin (wc = W_lin @ W_att blocks) and is
    computed TRANSPOSED per 256-row tile: attT[16, 256] = watt16.T @ xT,
    so the softmax axis is a free dim (no cross-partition reductions).
    The mask is injected in LOGIT space pre-LeakyReLU via a rank-1 bf16
    matmul accumulate of {0,-1e30} rows (exp == 0 exactly, matching the
    reference's post-leaky -1e9).
  * fp16 matmul inputs (1 cyc/row on PE; fp32 runs ~4 cyc/row) and fp16
    output DMA (host upcasts) — halves both DMA directions. ~5e-4 rel err.
  * MEGA-tiling: 16 tiles form one mega. All PE front matmuls run first,
    then ONE set of attention-chain ops processes all 16 tiles at once
    with tiles packed 4-per-32-partition-group ([128, 1024] slabs instead
    of [8, 256] slivers — DVE cost scales with free size only), then the
    16 back-ends (aw transpose + fused relu*aw + store). This keeps the
    PE queue free of long-latency waits (back-to-back matmuls stay
    pipelined) and amortizes per-instruction overhead 16x.

Sharding: pure data-parallel over batch: 512 batch rows per core
(128 tiles of 256 rows), weights replicated.
"""

import os
from contextlib import ExitStack

import ml_dtypes
import numpy as np

import concourse.bacc as bacc
import concourse.bass as bass
import concourse.tile as tile
from concourse import mybir
from concourse.bass_utils import run_bass_kernel_spmd

B, N, F = 4096, 64, 128
H, D = 8, 64
HD = H * D  # 512
NCORES = 8
BSHARD = B // NCORES  # 512
ROWS = BSHARD * N  # 32768
DT_ROWS = 256  # rows per tile (4 batch elements)
DTILES = ROWS // DT_ROWS  # 128
MEGA = 16  # tiles per mega (4 partition groups x 4 slots)

f32 = mybir.dt.float32
bf16 = mybir.dt.bfloat16
f16 = mybir.dt.float16

LAST_RESULT = None  # test harness reads exec_time_ns / trace from here


def build_nc(dtiles: int = DTILES) -> bass.Bass:
    nc = bacc.Bacc("TRN2", target_bir_lowering=False, debug=False)
    rows = dtiles * DT_ROWS
    assert dtiles % MEGA == 0

    xt = nc.declare_dram_parameter("xt", [dtiles, F, DT_ROWS], f16, isOutput=False)
    wlin_d = nc.declare_dram_parameter("wlin", [F, HD], f16, isOutput=False)
    watt_d = nc.declare_dram_parameter("watt", [F, 16], f16, isOutput=False)
    ident_d = nc.declare_dram_parameter("ident8", [128, 8], f16, isOutput=False)
    nmega = dtiles // MEGA
    maskslab_d = nc.declare_dram_parameter(
        "maskslab", [128, nmega * MEGA * N * 4 // 4], bf16, isOutput=False
    )
    out = nc.declare_dram_parameter("out", [rows, HD], f16, isOutput=True)

    mult = mybir.AluOpType.mult
    mmax = mybir.AluOpType.max

    with tile.TileContext(nc) as tc, ExitStack() as ctx:
        consts = ctx.enter_context(tc.tile_pool(name="consts", bufs=1))
        xin = ctx.enter_context(tc.tile_pool(name="xin", bufs=20))
        outp = ctx.enter_context(tc.tile_pool(name="outp", bufs=4))
        small = ctx.enter_context(tc.tile_pool(name="small", bufs=4))
        awsb = ctx.enter_context(tc.tile_pool(name="awsb", bufs=20))
        plin = ctx.enter_context(tc.tile_pool(name="plin", bufs=2, space="PSUM"))
        pattA = ctx.enter_context(tc.tile_pool(name="pattA", bufs=1, space="PSUM"))
        pattB = ctx.enter_context(tc.tile_pool(name="pattB", bufs=1, space="PSUM"))
        paw = ctx.enter_context(tc.tile_pool(name="paw", bufs=2, space="PSUM"))

        wlin_sb = consts.tile([F, HD], f16)
        nc.sync.dma_start(out=wlin_sb, in_=wlin_d[:])
        watt_sb = consts.tile([F, 16], f16)
        nc.sync.dma_start(out=watt_sb, in_=watt_d[:])
        # identity blocks replicated at partition bases 0/32/64/96 so the
        # aw transposes' fmap shares the weight operand's start partition
        ident_sb = consts.tile([128, 8], f16)
        nc.sync.dma_start(out=ident_sb, in_=ident_d[:])
        maskslab_sb = consts.tile([128, nmega * 1024], bf16)
        nc.sync.dma_start(out=maskslab_sb, in_=maskslab_d[:])

        # Persistent ping-pong mega slabs. Tiles pack 4-per-32-partition
        # group: tile i of a mega -> partition base 32*(i//4), free slot i%4.
        # memset once so untouched partitions stay finite for the sim.
        def mk_slabs(k):
            a = consts.tile([128, 4, DT_ROWS], f32, tag=f"slab_a{k}")
            s = consts.tile([128, 4, 4, 1], f32, tag=f"slab_s{k}")
            l = consts.tile([128, 4 * DT_ROWS], f32, tag=f"slab_l{k}")
            e = consts.tile([128, 4 * DT_ROWS], f32, tag=f"slab_e{k}")
            dn = consts.tile([128, 4, 4, 1], f32, tag=f"slab_dn{k}")
            rd = consts.tile([128, 4, 4, 1], f32, tag=f"slab_rd{k}")
            aw = consts.tile([128, 4 * DT_ROWS], f16, tag=f"slab_aw{k}")
            nc.vector.memset(a, 0.0)
            nc.vector.memset(s, 0.0)
            return a, s, l, e, dn, rd, aw

        slabs = [mk_slabs(0), mk_slabs(1)]

        def fronts(m):
            attA_m, src_m, attL_m, ew_m, den_m, rden_m, awT_m = slabs[m % 2]
            x_tiles = []
            # pair-level attT matmuls (N=512) + staging; lin runs in the
            # backs so its PSUM lifetime stays within one tile
            for j in range(MEGA // 2):
                i = 2 * j
                t = m * MEGA + i
                g, islot = i // 4, i % 4
                x2_sb = xin.tile([F, 2, DT_ROWS], f16, tag="x2")
                nc.sync.dma_start(
                    out=x2_sb,
                    in_=xt[t : t + 2].rearrange("two f r -> f two r"),
                )
                x_tiles.append(x2_sb[:, 0, :])
                x_tiles.append(x2_sb[:, 1, :])
                x_pair = x2_sb.rearrange("f two r -> f (two r)")

                # attA for both tiles of the pair (one clean accumulate
                # group with the logit-space mask), attB in its own bank.
                attA_ps = pattA.tile([8, 2, DT_ROWS], f32, tag="attA")
                nc.tensor.matmul(
                    attA_ps.rearrange("h two r -> h (two r)"),
                    watt_sb[:, 0:8],
                    x_pair,
                    start=True,
                    stop=True,
                )
                attB_ps = pattB.tile([8, 2, DT_ROWS], f32, tag="attB")
                nc.tensor.matmul(
                    attB_ps.rearrange("h two r -> h (two r)"),
                    watt_sb[:, 8:16],
                    x_pair,
                    start=True,
                    stop=True,
                )
                nc.scalar.copy(
                    out=attA_m[32 * g : 32 * g + 8, islot : islot + 2, :],
                    in_=attA_ps,
                )
                nc.scalar.copy(
                    out=src_m[32 * g : 32 * g + 8, islot : islot + 2, :, :],
                    in_=attB_ps.rearrange("h two (b n) -> h two b n", n=N)[
                        :, :, :, 0:1
                    ],
                )

            return x_tiles

        def chain(m):
            attA_m, src_m, attL_m, ew_m, den_m, rden_m, awT_m = slabs[m % 2]
            # one batched attention chain for the whole mega
            attS = attA_m.rearrange("p q (b n) -> p q b n", n=N)
            nc.vector.tensor_tensor(
                out=attS,
                in0=attS,
                in1=src_m.to_broadcast([128, 4, 4, N]),
                op=mybir.AluOpType.add,
            )
            nc.vector.tensor_tensor(
                out=attS,
                in0=attS,
                in1=maskslab_sb[:, m * 1024 : (m + 1) * 1024].rearrange(
                    "p (i b n) -> p i b n", i=4, b=4
                ),
                op=mybir.AluOpType.add,
            )
            nc.vector.scalar_tensor_tensor(
                out=attL_m.rearrange("p (q b n) -> p q b n", q=4, b=4),
                in0=attS,
                scalar=0.2,
                in1=attS,
                op0=mult,
                op1=mmax,
            )
            nc.scalar.activation(
                out=ew_m, in_=attL_m, func=mybir.ActivationFunctionType.Exp
            )
            nc.vector.tensor_reduce(
                out=den_m,
                in_=ew_m.rearrange("p (q b n) -> p q b n", q=4, b=4),
                axis=mybir.AxisListType.X,
                op=mybir.AluOpType.add,
            )
            nc.vector.reciprocal(rden_m, den_m)
            nc.vector.tensor_tensor(
                out=awT_m.rearrange("p (q b n) -> p q b n", q=4, b=4),
                in0=ew_m.rearrange("p (q b n) -> p q b n", q=4, b=4),
                in1=rden_m.to_broadcast([128, 4, 4, N]),
                op=mult,
            )

        def backs(m, x_tiles):
            attA_m, src_m, attL_m, ew_m, den_m, rden_m, awT_m = slabs[m % 2]
            # phase 1: all aw transposes back-to-back (single wait on the
            # chain), each staged to SBUF by ACT
            aw_sbs = []
            for i in range(MEGA):
                g, islot = i // 4, i % 4
                aw_ps = paw.tile([128, 16], f16, tag="aw_ps")
                for half in range(2):
                    nc.tensor.transpose(
                        aw_ps[:, half * 8 : half * 8 + 8],
                        awT_m[
                            32 * g : 32 * g + 8,
                            islot * DT_ROWS
                            + half * 128 : islot * DT_ROWS
                            + half * 128
                            + 128,
                        ],
                        ident_sb[32 * g : 32 * g + 8, :],
                        tile_position=(32 * g, 0),
                    )
                aw_sb = awsb.tile([128, 16], f16, tag="aw_sb")
                nc.scalar.copy(out=aw_sb, in_=aw_ps)
                aw_sbs.append(aw_sb)
            # phase 2: lin matmuls + fused relu(lin)*aw + paired stores
            for i in range(MEGA):
                t = m * MEGA + i
                if i % 2 == 0:
                    o2_sb = outp.tile([128, 2, 2, HD], f16, tag="o2")
                lin_ps = plin.tile([128, 2, HD], f32, tag="lin")
                nc.tensor.matmul(
                    lin_ps[:, 0, :],
                    x_tiles[i][:, 0:128],
                    wlin_sb,
                    start=True,
                    stop=True,
                )
                nc.tensor.matmul(
                    lin_ps[:, 1, :],
                    x_tiles[i][:, 128:256],
                    wlin_sb,
                    start=True,
                    stop=True,
                )
                nc.vector.scalar_tensor_tensor(
                    out=o2_sb[:, i % 2].rearrange("p two (h d) -> p (two h) d", h=H),
                    in0=lin_ps.rearrange("p two (h d) -> p (two h) d", h=H),
                    scalar=0.0,
                    in1=aw_sbs[i].to_broadcast([128, 2 * H, D]),
                    op0=mmax,
                    op1=mult,
                )
                if i % 2 == 1:
                    out_view = out[
                        (t - 1) * DT_ROWS : (t + 1) * DT_ROWS, :
                    ].rearrange("(four p) hd -> p four hd", four=4)
                    nc.sync.dma_start(
                        out=out_view,
                        in_=o2_sb.rearrange("p a b hd -> p (a b) hd"),
                    )

        # software-pipelined mega order: PE runs fronts(m+1) while the
        # DVE/ACT chain of mega m drains, then the backs of mega m
        nmega = dtiles // MEGA
        xt_prev = fronts(0)
        for m in range(nmega):
            chain(m)
            xt_next = fronts(m + 1) if m + 1 < nmega else None
            backs(m, xt_prev)
            xt_prev = xt_next

    nc.compile()
    return nc


def _host_weights(W_lin, W_att):
    W_lin64 = W_lin.astype(np.float64)
    wc2 = (W_lin64 @ W_att[HD:].astype(np.float64)).astype(np.float32)
    wc1 = (W_lin64 @ W_att[:HD].astype(np.float64)).astype(np.float32)
    watt16 = np.ascontiguousarray(
        np.concatenate([wc2, wc1], axis=1).astype(np.float16)
    )
    ident8 = np.zeros((128, 8), dtype=np.float16)
    for gg in range(4):
        ident8[32 * gg : 32 * gg + 8, :] = np.eye(8, dtype=np.float16)
    return W_lin.astype(np.float16), watt16, ident8


def _core_inputs(x_shard, mask_shard, wlin, watt16, ident8):
    nb = x_shard.shape[0]
    dtiles = nb * N // DT_ROWS
    xtv = np.ascontiguousarray(
        x_shard.reshape(dtiles, DT_ROWS, F).transpose(0, 2, 1).astype(np.float16)
    )
    mrow = np.where(mask_shard.reshape(-1) != 0, 0.0, -1e30).astype(np.float32)
    nmega = dtiles // MEGA
    # [m, g, islot, b, n] -> partition-group-replicated [128, nmega*1024]
    mr = mrow.reshape(nmega, MEGA, N * 4).reshape(nmega, 4, 4, N * 4)
    ms = mr.transpose(1, 0, 2, 3)  # [g, m, islot, (b n)]
    maskslab = (
        np.broadcast_to(ms[:, None], (4, 32, nmega, 4, N * 4))
        .reshape(128, nmega, 4 * N * 4)
        .reshape(128, nmega * 1024)
        .astype(ml_dtypes.bfloat16)
    )
    return {
        "xt": xtv,
        "wlin": wlin,
        "watt": watt16,
        "ident8": ident8,
        "maskslab": np.ascontiguousarray(maskslab),
    }


def kernel(x, W_lin, W_att, mask):
    global LAST_RESULT
    x = np.asarray(x, dtype=np.float32)
    W_lin = np.asarray(W_lin, dtype=np.float32)
    W_att = np.asarray(W_att, dtype=np.float32)
    mask = np.asarray(mask)

    wlin, watt16, ident8 = _host_weights(W_lin, W_att)
    in_maps = []
    for c in range(NCORES):
        in_maps.append(
            _core_inputs(
                x[c * BSHARD : (c + 1) * BSHARD],
                mask[c * BSHARD : (c + 1) * BSHARD],
                wlin,
                watt16,
                ident8,
            )
        )

    nc = build_nc(DTILES)
    trace = os.environ.get("KERNEL_TRACE", "0") == "1"
    tmpdir = os.environ.get("KERNEL_TRACE_DIR") or None
    res = run_bass_kernel_spmd(
        nc, in_maps, list(range(NCORES)), trace=trace, tmpdir=tmpdir
    )
    LAST_RESULT = res
    return np.concatenate(
        [
            res.results[c]["out"].astype(np.float32).reshape(BSHARD, N, HD)
            for c in range(NCORES)
        ],
        axis=0,
    )
